# revision 8
# baseline (speedup 1.0000x reference)
"""GATv2 message-passing kernel for 8 Trainium2 NeuronCores (v3).

Sharding: nodes split into 8 contiguous ranges; each edge belongs to the core
owning its dst node.  The tiny [500,16]-pooled head (mean + 3-layer MLP)
finishes on host.

Design notes (from v1/v2 trace analysis):
- dma_gather descriptor generation costs ~7.76 ns/index on a Q7 core pair and
  is the dominant cost.  Gathers are issued on SWDGE queues 1-3 (queue q runs
  on Q7 cores 2q/2q+1): they retire from the GpSimd engine in ~60 ns and
  generate asynchronously, 3 pairs in parallel.  Queue 0 would block the
  engine for the full generation time.  Only zl = xl[src] is gathered
  (256B rows); zr = xr[dst] is block-local and produced by the tensor engine
  as onehot^T @ xr_block.
- Onehot matrices (oh: edge-major, ohT: node-major) are precomputed on HOST
  and streamed in by HWDGE DMA - the DVE is_equal build (8.7 ps/elem),
  PE transpose, and ACT copy they replace were co-critical in v2.
- leaky relu runs on the ACT engine as Prelu(alpha=0.2) directly off PSUM
  (verified exact); DVE keeps only: m = lk*att (2x bf16 mode), the per-head
  reduce (fp16 out, 2x mode), and pz = p*zsum (PSUM read).
- z-sum trick: sum_e a_e*(zl+zr) = S + xr[dst] since softmax weights sum to
  1, so -xr is folded into the residual weights (Wres - Wr, bias - br).
- Per dst-block variable chunk counts (max across cores, shared SPMD
  program); appended self-loops are a per-block "self chunk" whose onehot is
  the identity and whose zsum = (xl+xr)[block] is precomputed in phase A.
"""

import os
from contextlib import ExitStack

import numpy as np
import ml_dtypes

N_NODES = 50000
IN_CH = 64
HEADS = 8
OUT_CH = 16
HID = 128
N_GRAPHS = 500
NEG = 0.2

N_CORES = 8
NPC = N_NODES // N_CORES          # 6250
P = 128
NBLK = (NPC + P - 1) // P         # 49
NSLOT = NBLK * P                  # 6272
R = 136                           # rhs cols: 128 pz + 8 p
SPLIT = 32768
NROWS_A = SPLIT
NROWS_B = ((N_NODES + P - 1) // P) * P - SPLIT   # 17408
GB = 2                            # blocks per gather/onehot group
WGC = 4                           # chunks per compute batch

bf16 = ml_dtypes.bfloat16

_CACHE = {}


def _wrap_idx(flat):
    """int16 index list -> [128, n/16] (16-wrapped, replicated per Q7 core)."""
    w = flat.reshape(-1, 16).T.astype(np.int16)   # [16, n/16]
    return np.tile(w, (8, 1)).copy()


def _host_prep(x, edge_index, batch, Wl, bl, Wr, br, att, Wres, bias, Wlin,
               blin):
    x = np.asarray(x, np.float32)
    ei = np.asarray(edge_index).astype(np.int64)
    batch = np.asarray(batch).astype(np.int64)

    src_all = ei[0]
    dst_all = ei[1]

    WlE1 = np.concatenate([Wl, np.asarray(bl, np.float32)[None, :]], 0)
    WrE1 = np.concatenate([Wr, np.asarray(br, np.float32)[None, :]], 0)
    WsE1 = WlE1 + WrE1
    Wresr1 = np.concatenate([np.asarray(Wres, np.float32) - Wr,
                             (np.asarray(bias, np.float32) -
                              np.asarray(br, np.float32))[None, :]], 0)

    attb = np.broadcast_to(np.asarray(att, np.float32).reshape(-1).astype(bf16),
                           (P, HID)).copy()
    ident = np.eye(P, dtype=np.float32).astype(bf16)

    NROWS_L = NROWS_A + NROWS_B
    xT1_full = np.zeros((IN_CH + 1, NROWS_L), np.float32)
    xT1_full[:IN_CH, :N_NODES] = x.T
    xT1_full[IN_CH, :] = 1.0

    core_of = (dst_all // NPC).astype(np.int32)
    percore = []
    nL = np.zeros((N_CORES, NBLK), np.int64)
    nH = np.zeros((N_CORES, NBLK), np.int64)
    for c in range(N_CORES):
        sel = np.nonzero(core_of == c)[0]
        srcs = src_all[sel]
        dloc = (dst_all[sel] - c * NPC).astype(np.int64)
        blk = dloc // P
        hi = (srcs >= SPLIT).astype(np.int64)
        order = np.lexsort((hi, blk))
        srcs, dloc, blk, hi = (a[order] for a in (srcs, dloc, blk, hi))
        nL[c] = np.bincount(blk[hi == 0], minlength=NBLK)
        nH[c] = np.bincount(blk[hi == 1], minlength=NBLK)
        percore.append((srcs, dloc, blk, hi))

    # uniform (max over cores) chunk counts per block for the SPMD program
    KL = ((nL.max(0) + P - 1) // P).astype(np.int64)
    KH = ((nH.max(0) + P - 1) // P).astype(np.int64)

    gmin = np.empty(N_CORES, np.int64)
    gmax = np.empty(N_CORES, np.int64)
    for c in range(N_CORES):
        bs = batch[c * NPC:min((c + 1) * NPC, N_NODES)]
        gmin[c] = bs[0]
        gmax[c] = bs[-1]
    span = int((gmax - gmin).max()) + 1
    W = min(max(-(-span // P) * P, P), 512)

    KLsum, KHsum = int(KL.sum()), int(KH.sum())
    NCH_TOT = KLsum + KHsum + NBLK        # + self chunk per block
    offL = np.concatenate([[0], np.cumsum(KL)])
    offH = np.concatenate([[0], np.cumsum(KH)])

    arange_p = np.arange(P, dtype=np.float32)

    in_maps = []
    for c in range(N_CORES):
        srcs, dloc, blk, hi = percore[c]
        idxL = np.zeros(KLsum * P, np.int64)
        idxH = np.zeros(KHsum * P, np.int64)
        dstv = np.full((NCH_TOT, P), -1.0, np.float32)
        cum_nl = np.concatenate([[0], np.cumsum(nL[c] + nH[c])])
        gc = 0
        for b in range(NBLK):
            s0 = cum_nl[b]
            nl, nh = int(nL[c][b]), int(nH[c][b])
            eL = slice(s0, s0 + nl)
            eH = slice(s0 + nl, s0 + nl + nh)
            idxL[offL[b] * P:offL[b] * P + nl] = srcs[eL]
            idxH[offH[b] * P:offH[b] * P + nh] = srcs[eH] - SPLIT
            dstv[gc:gc + KL[b]].reshape(-1)[:nl] = (dloc[eL] -
                                                    b * P).astype(np.float32)
            gc += int(KL[b])
            dstv[gc:gc + KH[b]].reshape(-1)[:nh] = (dloc[eH] -
                                                    b * P).astype(np.float32)
            gc += int(KH[b])
            dstv[gc] = arange_p                       # self chunk
            gc += 1
        assert gc == NCH_TOT

        # onehots: oh[gc, p_edge, n] ; ohT = transpose
        oh_all = (dstv[:, :, None] == arange_p[None, None, :]).astype(bf16)
        oh_d = oh_all.transpose(1, 0, 2).reshape(P, NCH_TOT * P).copy()
        ohT_d = oh_all.transpose(2, 0, 1).reshape(P, NCH_TOT * P).copy()

        lo = c * NPC
        hicap = min((c + 1) * NPC, N_NODES)
        xT1c = np.zeros((IN_CH + 1, NSLOT), np.float32)
        xT1c[:IN_CH, :hicap - lo] = x[lo:hicap].T
        xT1c[IN_CH, :] = 1.0

        poh = np.zeros((NSLOT, W), np.float32)
        g = batch[lo:hicap] - gmin[c]
        poh[np.arange(hicap - lo), g] = 1.0

        in_maps.append({
            "xT1_full": xT1_full.astype(bf16),
            "xT1_core": xT1c.astype(bf16),
            "WlE1": WlE1.astype(bf16),
            "WrE1": WrE1.astype(bf16),
            "WsE1": WsE1.astype(bf16),
            "Wresr1": Wresr1.astype(bf16),
            "WlinB": np.asarray(Wlin, np.float32).astype(bf16),
            "blinB": np.broadcast_to(np.asarray(blin, np.float32),
                                     (P, OUT_CH)).copy(),
            "attb": attb, "ident": ident,
            "idxL": _wrap_idx(idxL), "idxH": _wrap_idx(idxH),
            "oh_d": oh_d, "ohT_d": ohT_d,
            "pool_oh": poh.astype(bf16),
        })

    counts = np.bincount(batch, minlength=N_GRAPHS).astype(np.float32)
    meta = dict(KL=tuple(int(v) for v in KL), KH=tuple(int(v) for v in KH),
                W=W, gmin=gmin, counts=counts)
    return in_maps, meta


def _build_program(KL, KH, W):
    import concourse.bass as bass
    import concourse.tile as tile
    from concourse import mybir, bacc

    fp32 = mybir.dt.float32
    bft = mybir.dt.bfloat16
    f16 = mybir.dt.float16
    i16 = mybir.dt.int16
    AF = mybir.ActivationFunctionType
    OP = mybir.AluOpType

    KL = np.asarray(KL, np.int64)
    KH = np.asarray(KH, np.int64)
    KLsum, KHsum = int(KL.sum()), int(KH.sum())
    NCH_TOT = KLsum + KHsum + NBLK
    NG = (NBLK + GB - 1) // GB
    offL = np.concatenate([[0], np.cumsum(KL)]).astype(int)
    offH = np.concatenate([[0], np.cumsum(KH)]).astype(int)
    # global chunk-column offset of block b's chunks: L at gcL[b], H at
    # gcL[b]+KL[b], self at gcL[b]+KL[b]+KH[b]
    gcB = np.concatenate([[0], np.cumsum(KL + KH + 1)]).astype(int)
    kwLg = [int(KL[g * GB:min((g + 1) * GB, NBLK)].sum()) for g in range(NG)]
    kwHg = [int(KH[g * GB:min((g + 1) * GB, NBLK)].sum()) for g in range(NG)]
    nchg = [int(gcB[min((g + 1) * GB, NBLK)] - gcB[g * GB])
            for g in range(NG)]
    KWL_MAX, KWH_MAX = max(kwLg), max(kwHg)
    NCHG_MAX = max(nchg)
    NROWS_L = NROWS_A + NROWS_B
    NXCH = NROWS_L // P

    nc = bacc.Bacc("TRN2", target_bir_lowering=False, debug=False,
                   num_devices=N_CORES, num_swdge_queues=4)

    def din(name, shape, dt):
        return nc.dram_tensor(name, shape, dt, kind="ExternalInput").ap()

    xT1_full = din("xT1_full", [IN_CH + 1, NROWS_L], bft)
    xT1_core = din("xT1_core", [IN_CH + 1, NSLOT], bft)
    WlE1 = din("WlE1", [IN_CH + 1, HID], bft)
    WrE1 = din("WrE1", [IN_CH + 1, HID], bft)
    WsE1 = din("WsE1", [IN_CH + 1, HID], bft)
    Wresr1 = din("Wresr1", [IN_CH + 1, HID], bft)
    WlinB = din("WlinB", [HID, OUT_CH], bft)
    blinB = din("blinB", [P, OUT_CH], fp32)
    attb = din("attb", [P, HID], bft)
    ident = din("ident", [P, P], bft)
    idxL = din("idxL", [P, KLsum * 8], i16)
    idxH = din("idxH", [P, KHsum * 8], i16)
    oh_d = din("oh_d", [P, NCH_TOT * P], bft)
    ohT_d = din("ohT_d", [P, NCH_TOT * P], bft)
    pool_oh = din("pool_oh", [NSLOT, W], bft)

    gpart = nc.dram_tensor("gpart", [OUT_CH, W], fp32,
                           kind="ExternalOutput").ap()
    DBG = bool(int(os.environ.get("KERNEL_DEBUG", "0")))
    if DBG:
        dbg_zs = nc.dram_tensor("dbg_zs", [P, WGC * HID], fp32,
                                kind="ExternalOutput").ap()
        dbg_lk = nc.dram_tensor("dbg_lk", [P, WGC * HID], fp32,
                                kind="ExternalOutput").ap()
        dbg_al = nc.dram_tensor("dbg_al", [P, WGC * HEADS], fp32,
                                kind="ExternalOutput").ap()
        dbg_rhs = nc.dram_tensor("dbg_rhs", [P, 24 * R], fp32,
                                 kind="ExternalOutput").ap()
        dbg_pu = nc.dram_tensor("dbg_pu", [P, R], fp32,
                                kind="ExternalOutput").ap()

    tabA = nc.dram_tensor("tabA", [NROWS_A, HID], bft).ap()
    tabB = nc.dram_tensor("tabB", [NROWS_B, HID], bft).ap()

    with tile.TileContext(nc) as tc, ExitStack() as ctx:
        res = ctx.enter_context(tc.tile_pool(name="res", bufs=1))
        xT1c_t = res.tile([IN_CH + 1, NSLOT], bft)
        nc.sync.dma_start(xT1c_t[:], xT1_core[:])
        WlE1_t = res.tile([IN_CH + 1, HID], bft)
        nc.sync.dma_start(WlE1_t[:], WlE1[:])
        WrE1_t = res.tile([IN_CH + 1, HID], bft)
        nc.sync.dma_start(WrE1_t[:], WrE1[:])
        WsE1_t = res.tile([IN_CH + 1, HID], bft)
        nc.sync.dma_start(WsE1_t[:], WsE1[:])
        Wresr1_t = res.tile([IN_CH + 1, HID], bft)
        nc.sync.dma_start(Wresr1_t[:], Wresr1[:])
        Wlin_t = res.tile([HID, OUT_CH], bft)
        nc.sync.dma_start(Wlin_t[:], WlinB[:])
        blin_t = res.tile([P, OUT_CH], fp32)
        nc.sync.dma_start(blin_t[:], blinB[:])
        attb_t = res.tile([P, HID], bft)
        nc.sync.dma_start(attb_t[:], attb[:])
        id_t = res.tile([P, P], bft)
        nc.sync.dma_start(id_t[:], ident[:])
        idxL_t = res.tile([P, KLsum * 8], i16)
        nc.sync.dma_start(idxL_t[:], idxL[:])
        idxH_t = res.tile([P, KHsum * 8], i16)
        nc.sync.dma_start(idxH_t[:], idxH[:])
        xr_core = res.tile([P, NBLK, HID], bft)
        xl_core = res.tile([P, NBLK, HID], bft)

        # ---------------- phase A: tables + xr/zs cores -------------------
        XSL = 49
        with tc.tile_pool(name="pa_sb", bufs=3) as pa_sb, \
             tc.tile_pool(name="pa_x", bufs=2) as pa_x, \
             tc.tile_pool(name="pa_ps", bufs=2, space="PSUM") as pa_ps:
            for s0 in range(0, NXCH, XSL):
                s1 = min(s0 + XSL, NXCH)
                xs = pa_x.tile([IN_CH + 1, XSL * P], bft, tag="xs")
                nc.sync.dma_start(xs[:, 0:(s1 - s0) * P],
                                  xT1_full[:, s0 * P:s1 * P])
                for i in range(s0, s1):
                    j = i - s0
                    ps = pa_ps.tile([P, HID], fp32, space="PSUM", tag="ps")
                    nc.tensor.matmul(ps[:], lhsT=xs[:, j * P:(j + 1) * P],
                                     rhs=WlE1_t[:], start=True, stop=True)
                    sb = pa_sb.tile([P, HID], bft, tag="sb")
                    if i % 2 == 0:
                        nc.scalar.copy(sb[:], ps[:])
                    else:
                        nc.vector.tensor_copy(sb[:], ps[:])
                    if i * P < NROWS_A:
                        nc.sync.dma_start(tabA[i * P:(i + 1) * P, :], sb[:])
                    else:
                        r0 = i * P - NROWS_A
                        nc.sync.dma_start(tabB[r0:r0 + P, :], sb[:])
            for b in range(NBLK):
                ps = pa_ps.tile([P, HID], fp32, space="PSUM", tag="ps")
                nc.tensor.matmul(ps[:], lhsT=xT1c_t[:, b * P:(b + 1) * P],
                                 rhs=WrE1_t[:], start=True, stop=True)
                if b % 2 == 0:
                    nc.scalar.copy(xr_core[:, b, :], ps[:])
                else:
                    nc.vector.tensor_copy(xr_core[:, b, :], ps[:])
                ps2 = pa_ps.tile([P, HID], fp32, space="PSUM", tag="ps")
                nc.tensor.matmul(ps2[:], lhsT=xT1c_t[:, b * P:(b + 1) * P],
                                 rhs=WlE1_t[:], start=True, stop=True)
                if b % 2 == 1:
                    nc.scalar.copy(xl_core[:, b, :], ps2[:])
                else:
                    nc.vector.tensor_copy(xl_core[:, b, :], ps2[:])

        # ---------------- phase B ----------------------------------------
        zL_pool = ctx.enter_context(tc.tile_pool(name="zL", bufs=4))
        zH_pool = ctx.enter_context(tc.tile_pool(name="zH", bufs=4))
        rhs_pool = ctx.enter_context(tc.tile_pool(name="rhs", bufs=2))
        oh_pool = ctx.enter_context(tc.tile_pool(name="ohp", bufs=2))
        ohT_pool = ctx.enter_context(tc.tile_pool(name="ohTp", bufs=2))
        m_pool = ctx.enter_context(tc.tile_pool(name="m", bufs=2))
        blk_pool = ctx.enter_context(tc.tile_pool(name="blk", bufs=2))
        poh_pool = ctx.enter_context(tc.tile_pool(name="poh", bufs=2))
        zs_ps = ctx.enter_context(tc.tile_pool(name="zs4", bufs=2,
                                               space="PSUM"))
        pu_ps = ctx.enter_context(tc.tile_pool(name="pu", bufs=2,
                                               space="PSUM"))
        pf_ps = ctx.enter_context(tc.tile_pool(name="pf", bufs=1,
                                               space="PSUM"))
        pt_ps = ctx.enter_context(tc.tile_pool(name="ptt", bufs=1,
                                               space="PSUM"))
        pg_ps = ctx.enter_context(tc.tile_pool(name="pg", bufs=1,
                                               space="PSUM"))

        pg = pg_ps.tile([OUT_CH, W], fp32, space="PSUM")

        qctr = 0
        for g in range(NG):
            b0, b1 = g * GB, min((g + 1) * GB, NBLK)
            kwL, kwH = kwLg[g], kwHg[g]
            ztL = zL_pool.tile([P, KWL_MAX, HID], bft, tag="ztL", name="ztL")
            ztH = zH_pool.tile([P, KWH_MAX, HID], bft, tag="ztH", name="ztH")
            if kwL:
                nc.gpsimd.dma_gather(
                    out_ap=ztL[:, 0:kwL, :], in_ap=tabA[:],
                    idxs_ap=idxL_t[:, offL[b0] * 8:(offL[b0] + kwL) * 8],
                    num_idxs=kwL * P, num_idxs_reg=kwL * P, elem_size=HID,
                    single_packet=False, queue_num=1 + qctr % 3)
                qctr += 1
            if kwH:
                nc.gpsimd.dma_gather(
                    out_ap=ztH[:, 0:kwH, :], in_ap=tabB[:],
                    idxs_ap=idxH_t[:, offH[b0] * 8:(offH[b0] + kwH) * 8],
                    num_idxs=kwH * P, num_idxs_reg=kwH * P, elem_size=HID,
                    single_packet=False, queue_num=1 + qctr % 3)
                qctr += 1

            ng = nchg[g]
            gch0 = gcB[b0]
            oh_t = oh_pool.tile([P, NCHG_MAX, P], bft, tag="oh", name="oh_t")
            nc.sync.dma_start(oh_t[:, 0:ng, :],
                              oh_d[:, gch0 * P:(gch0 + ng) * P])
            ohT_t = ohT_pool.tile([P, NCHG_MAX, P], bft, tag="ohT",
                                  name="ohT_t")
            nc.sync.dma_start(ohT_t[:, 0:ng, :],
                              ohT_d[:, gch0 * P:(gch0 + ng) * P])
            rhs = rhs_pool.tile([P, NCHG_MAX, R], bft, tag="rhs", name="rhs")

            for b in range(b0, b1):
                # chunk list: (kind, zt-slot or zs col)
                chunks = ([("L", offL[b] - offL[b0] + j)
                           for j in range(int(KL[b]))] +
                          [("H", offH[b] - offH[b0] + j)
                           for j in range(int(KH[b]))] +
                          [("S", b)])
                rc0 = int(gcB[b] - gch0)       # chunk col within group tiles
                nchb = len(chunks)
                pu = pu_ps.tile([P, R], fp32, space="PSUM", tag="pu",
                                name="pu")
                ci = 0
                for w0 in range(0, nchb, WGC):
                    w1 = min(w0 + WGC, nchb)
                    nb = w1 - w0
                    batch = chunks[w0:w1]
                    zs4 = zs_ps.tile([P, WGC, HID], fp32, space="PSUM",
                                     tag="zs4", name="zs4")
                    # zr matmuls (all chunks; self's ohT slice is identity)
                    # NOTE: start=True clears has_written for the WHOLE PSUM
                    # bank, so only the first matmul of the batch starts; the
                    # rest write to cleared (has_written=0) elements, which
                    # is overwrite semantics.
                    for j, (kind, slot) in enumerate(batch):
                        nc.tensor.matmul(zs4[:, j, :],
                                         lhsT=ohT_t[:, rc0 + w0 + j, :],
                                         rhs=xr_core[:, b, :],
                                         start=(j == 0), stop=False,
                                         skip_group_check=True)
                    # zl adds: runs of consecutive same-stream chunks get one
                    # wide matmul with the shared identity stationary
                    ri = 0
                    while ri < nb:
                        kind, slot = batch[ri]
                        if kind == "S":
                            nc.tensor.matmul(zs4[:, ri, :], lhsT=id_t[:],
                                             rhs=xl_core[:, slot, :],
                                             start=False, stop=True,
                                             skip_group_check=True)
                            ri += 1
                            continue
                        rj = ri
                        while (rj + 1 < nb and batch[rj + 1][0] == kind and
                               batch[rj + 1][1] == batch[rj][1] + 1):
                            rj += 1
                        zt = ztL if kind == "L" else ztH
                        nc.tensor.matmul(
                            zs4[:, ri:rj + 1, :], lhsT=id_t[:],
                            rhs=zt[:, slot:slot + (rj - ri + 1), :],
                            start=False, stop=True, skip_group_check=True)
                        ri = rj + 1
                    if DBG and b == 0 and w0 == 0:
                        dzs = res.tile([P, WGC * HID], fp32, name="dzs")
                        nc.vector.tensor_copy(
                            dzs[:], zs4[:].rearrange("p a b -> p (a b)"))
                        nc.sync.dma_start(dbg_zs[:], dzs[:])
                    lk4 = m_pool.tile([P, WGC, HID], bft, tag="lk4",
                                      name="lk4")
                    nc.scalar.activation(lk4[:, 0:nb, :], zs4[:, 0:nb, :],
                                         AF.Prelu, alpha=NEG)
                    if DBG and b == 0 and w0 == 0:
                        dlk = res.tile([P, WGC * HID], fp32, name="dlk")
                        nc.vector.tensor_copy(
                            dlk[:], lk4[:].rearrange("p a b -> p (a b)"))
                        nc.sync.dma_start(dbg_lk[:], dlk[:])
                    m4 = m_pool.tile([P, WGC, HID], bft, tag="m4", name="m4")
                    nc.vector.tensor_tensor(
                        out=m4[:, 0:nb, :], in0=lk4[:, 0:nb, :],
                        in1=attb_t[:].rearrange("p (w h) -> p w h", w=1)
                            .to_broadcast([P, nb, HID]),
                        op=OP.mult)
                    alph = m_pool.tile([P, WGC, HEADS], fp32, tag="alph",
                                       name="alph")
                    nc.vector.tensor_reduce(
                        out=alph[:, 0:nb, :],
                        in_=m4[:, 0:nb, :].rearrange(
                            "p w (h c) -> p w h c", c=OUT_CH),
                        axis=mybir.AxisListType.X, op=OP.add)
                    if DBG and b == 0 and w0 == 0:
                        dal = res.tile([P, WGC * HEADS], fp32, name="dal")
                        nc.vector.tensor_copy(
                            dal[:], alph[:].rearrange("p a b -> p (a b)"))
                        nc.sync.dma_start(dbg_al[:], dal[:])
                    nc.scalar.activation(rhs[:, rc0 + w0:rc0 + w1, HID:R],
                                         alph[:, 0:nb, :], AF.Exp)
                    nc.vector.tensor_tensor(
                        out=rhs[:, rc0 + w0:rc0 + w1, 0:HID].rearrange(
                            "p w (h c) -> p w h c", c=OUT_CH),
                        in0=zs4[:, 0:nb, :].rearrange("p w (h c) -> p w h c",
                                                      c=OUT_CH),
                        in1=rhs[:, rc0 + w0:rc0 + w1, HID:R].to_broadcast(
                            [P, nb, HEADS, OUT_CH]),
                        op=OP.mult)
                    for j in range(nb):
                        nc.tensor.matmul(pu[:],
                                         lhsT=oh_t[:, rc0 + w0 + j, :],
                                         rhs=rhs[:, rc0 + w0 + j, :],
                                         start=(ci == 0),
                                         stop=(ci == nchb - 1))
                        ci += 1

                if DBG and b == 0:
                    drh = res.tile([P, 24 * R], fp32, name="drh")
                    nc.vector.tensor_copy(
                        drh[:, 0:nchb * R],
                        rhs[:, rc0:rc0 + nchb, :].rearrange(
                            "p a b -> p (a b)"))
                    nc.sync.dma_start(dbg_rhs[:, 0:nchb * R],
                                      drh[:, 0:nchb * R])
                    dpu = res.tile([P, R], fp32, name="dpu")
                    nc.vector.tensor_copy(dpu[:], pu[:])
                    nc.sync.dma_start(dbg_pu[:], dpu[:])
                # ---------------- block tail -----------------------------
                pr = pf_ps.tile([P, HID], fp32, space="PSUM", tag="pr",
                                name="pr")
                nc.tensor.matmul(pr[:], lhsT=xT1c_t[:, b * P:(b + 1) * P],
                                 rhs=Wresr1_t[:], start=True, stop=True)
                pu_sb = blk_pool.tile([P, R], fp32, tag="pu_sb",
                                      name="pu_sb")
                nc.scalar.copy(pu_sb[:], pu[:])
                pr_sb = blk_pool.tile([P, HID], fp32, tag="pr_sb",
                                      name="pr_sb")
                nc.scalar.copy(pr_sb[:], pr[:])
                den = blk_pool.tile([P, HEADS], fp32, tag="den", name="den")
                nc.vector.tensor_scalar(out=den[:], in0=pu_sb[:, HID:R],
                                        scalar1=1e-12, scalar2=None,
                                        op0=OP.max)
                rec = blk_pool.tile([P, HEADS], fp32, tag="rec", name="rec")
                nc.vector.reciprocal(rec[:], den[:])
                uo = blk_pool.tile([P, HID], fp32, tag="uo", name="uo")
                nc.vector.tensor_tensor(
                    out=uo[:].rearrange("p (h c) -> p h c", c=OUT_CH),
                    in0=pu_sb[:, 0:HID].rearrange("p (h c) -> p h c",
                                                  c=OUT_CH),
                    in1=rec[:].to_broadcast([P, HEADS, OUT_CH]), op=OP.mult)
                op_t = blk_pool.tile([P, HID], bft, tag="op", name="op_t")
                nc.vector.tensor_add(op_t[:], uo[:], pr_sb[:])
                ptt = pt_ps.tile([P, P], bft, space="PSUM", tag="ptt",
                                 name="ptt")
                nc.tensor.transpose(ptt[:], op_t[:], id_t[:])
                opT = blk_pool.tile([P, P], bft, tag="opT", name="opT")
                nc.scalar.copy(opT[:], ptt[:])
                phm = pf_ps.tile([P, OUT_CH], fp32, space="PSUM", tag="phm",
                                 name="phm")
                nc.tensor.matmul(phm[:], lhsT=opT[:], rhs=Wlin_t[:],
                                 start=True, stop=True)
                v = blk_pool.tile([P, OUT_CH], fp32, tag="v", name="v")
                nc.vector.tensor_add(v[:], phm[:], blin_t[:])
                rl = blk_pool.tile([P, OUT_CH], fp32, tag="rl", name="rl")
                nc.scalar.activation(rl[:], v[:], AF.Relu)
                ex = blk_pool.tile([P, OUT_CH], fp32, tag="ex", name="ex")
                nc.scalar.activation(ex[:], v[:], AF.Exp)
                # h_emit = relu(v) + min(exp(v), 1) = elu(v) + 1; the +1 per
                # node is subtracted on host via the per-graph counts
                h = blk_pool.tile([P, OUT_CH], bft, tag="h", name="h")
                nc.vector.scalar_tensor_tensor(out=h[:], in0=ex[:],
                                               scalar=1.0, op0=OP.min,
                                               op1=OP.add, in1=rl[:])
                poh_b = poh_pool.tile([P, W], bft, tag="poh", name="poh_b")
                nc.sync.dma_start(poh_b[:], pool_oh[b * P:(b + 1) * P, :])
                nc.tensor.matmul(pg[:], lhsT=h[:], rhs=poh_b[:],
                                 start=(b == 0), stop=(b == NBLK - 1))

        gout = res.tile([OUT_CH, W], fp32)
        nc.vector.tensor_copy(gout[:], pg[:])
        nc.sync.dma_start(gpart[:], gout[:])

    nc.compile()
    return nc


def kernel(x, edge_index, batch, Wl, bl, Wr, br, att, Wres, bias, Wlin, blin,
           W1, b1, W2, b2, W3, b3):
    from concourse.bass_utils import run_bass_kernel_spmd

    in_maps, meta = _host_prep(x, edge_index, batch, Wl, bl, Wr, br, att,
                               Wres, bias, Wlin, blin)
    key = (meta["KL"], meta["KH"], meta["W"])
    if key not in _CACHE:
        _CACHE[key] = _build_program(*key)
    nc = _CACHE[key]

    trace = bool(int(os.environ.get("KERNEL_TRACE", "0")))
    res = run_bass_kernel_spmd(nc, in_maps, list(range(N_CORES)),
                               trace=trace)
    if trace and res.exec_time_ns is not None:
        kernel.last_exec_ns = res.exec_time_ns
        kernel.last_mean_exec_ns = res.mean_exec_time_ns
        kernel.last_res = res

    G = np.zeros((N_GRAPHS, OUT_CH), np.float32)
    gmin = meta["gmin"]
    W = meta["W"]
    for c in range(N_CORES):
        gp = res.results[c]["gpart"].astype(np.float32)
        lo = int(gmin[c])
        hi = min(lo + W, N_GRAPHS)
        G[lo:hi] += gp.T[: hi - lo]
    G = G - meta["counts"][:, None]      # h_emit = elu + 1 on device
    g = G / np.maximum(meta["counts"], 1.0)[:, None]
    g = np.maximum(g @ np.asarray(W1, np.float32) + np.asarray(b1, np.float32), 0.0)
    g = np.maximum(g @ np.asarray(W2, np.float32) + np.asarray(b2, np.float32), 0.0)
    return (g @ np.asarray(W3, np.float32) + np.asarray(b3, np.float32)).astype(np.float32)
